# revision 4
# baseline (speedup 1.0000x reference)
"""Modulated deformable conv (DCNv2-style) Trainium2 Bass kernel.

Batch data-parallel over 8 NeuronCores (1 batch element per core).

Per-core pipeline:
  1. fuse 1x1 conv (PE)  -> x, kept as CHW padded in SBUF (X2, with a
     col-shifted duplicate on partitions 64-127 for K-stacked conv taps)
     and as NHWC 2-pixel tokens in HBM (xtok) for gathering.
  2. dy/dx/mod 3x3 convs (PE, 3 pair-slots K=128 + 3 single-slots K=64).
  3. map pipeline (DVE/ACT): floor flags, fracs, modulated corner-weight
     maps CY0/CY1, int16 gather token indices.
  4. dma_gather (transpose=True, 256B tokens = 2px x 64ch bf16): for each
     kernel tap k and corner row y, V[128=(2px,64ch), npix].
  5. corner weights replicated across 128 partitions via PE selector
     matmuls; prod = V * Wrep (DVE); final contraction on PE with w_reg
     folded lhsT (sign/2x-sigmoid folded in host-side).

Column ordering note: gather list position n maps to map-column
sigma(n) = (n%16)*S + n//16  (S = chunk_cols/16) because indices are
stored 16-partition-wrapped with a contiguous inner dim. sigma is applied
at 3 AP sites: the index wrap DMA, the repl-matmul rhs view, and the
final PSUM->SBUF unpermute copy.
"""
import sys

sys.path.insert(0, "/opt/trn_rl_repo")

from contextlib import ExitStack

import numpy as np

import concourse.bass as bass
import concourse.bacc as bacc
import concourse.mybir as mybir
from concourse.tile import TileContext
from concourse.mybir import AluOpType as Op
from concourse.mybir import ActivationFunctionType as Act

F32 = mybir.dt.float32
BF16 = mybir.dt.bfloat16
I16 = mybir.dt.int16

W = 128
C = 64
K2 = 9
PADW = 132


def _shape_consts(H):
    NPX = H * W
    NCH = 8
    CH = NPX // NCH          # pixels per chunk
    RPC = CH // W            # image rows per chunk
    S = CH // 16             # wrap cols per chunk slot
    PADH = H + 4
    NTOK = PADH * PADW
    NSPL = max(1, CH // 512)
    SPL = CH // NSPL         # matmul N per split (<=512)
    return NPX, NCH, CH, RPC, S, PADH, NTOK, NSPL, SPL


def build_nc(H=128, num_devices=8):
    NPX, NCH, CH, RPC, S, PADH, NTOK, NSPL, SPL = _shape_consts(H)
    RSPL = SPL // W                      # image rows per split
    NTOKP = ((NTOK + 2047) // 2048) * 2048

    es = ExitStack()
    nc = bacc.Bacc("TRN2", target_bir_lowering=False, debug=False,
                   num_devices=num_devices)

    x_img = nc.dram_tensor("x_img", [C, NPX], F32, kind="ExternalInput")
    x_cont = nc.dram_tensor("x_cont", [C, NPX], F32, kind="ExternalInput")
    out = nc.dram_tensor("out", [C, NPX], F32, kind="ExternalOutput")

    fuse_lhsT = nc.dram_tensor("fuse_lhsT", [128, 64], BF16, kind="ExternalInput")
    # conv lhsTs: per (q, v) where v = input-row type (6 rows feeding a
    # 4-output-row block); M-cols = delta*32 + lane (18 dup-lanes used)
    pw, sw = {}, {}
    for q in ("dy", "dx", "mod"):
        for v in range(6):
            pw[(q, v)] = nc.dram_tensor(f"pw_{q}_{v}", [128, 128], BF16,
                                        kind="ExternalInput")
            sw[(q, v)] = nc.dram_tensor(f"sw_{q}_{v}", [64, 128], BF16,
                                        kind="ExternalInput")
    dcols = {}
    for nm in ("bias_dy", "bias_dx", "bias_mod", "xw_s1", "xw_s2"):
        dcols[nm] = nc.dram_tensor(nm, [128, 1], F32, kind="ExternalInput")
    btg = {}
    for g in range(2):
        btg[("bt0", g)] = nc.dram_tensor(f"bt0_{g}", [128, CH], F32,
                                         kind="ExternalInput")
        btg[("btd", g)] = nc.dram_tensor(f"btd_{g}", [128, CH], F32,
                                         kind="ExternalInput")
    sel = nc.dram_tensor("sel", [128, K2 * 128], BF16, kind="ExternalInput")
    reg0 = nc.dram_tensor("reg0", [128, K2 * 64], BF16, kind="ExternalInput")
    reg1 = nc.dram_tensor("reg1", [128, K2 * 64], BF16, kind="ExternalInput")

    xtok = nc.dram_tensor("xtok", [NTOKP, 128], BF16, kind="Internal")
    # column-major token image: row (x, y) so a y-pair is 512B contiguous
    xtokC3 = xtok.ap()[0:NTOK, :].rearrange("(a b) e -> a b e", b=PADH)
    NPAIR_P = PADH // 2
    NPAIR_Q = PADH // 2 - 1
    QBASE = NPAIR_P * PADW
    NTOK2 = (NPAIR_P + NPAIR_Q) * PADW
    xtok2 = nc.dram_tensor("xtok2", [NTOK2, 256], BF16, kind="Internal")

    MM = lambda *a, **k: nc.tensor.matmul(*a, **k)

    with TileContext(nc) as tc:
        pconst = es.enter_context(tc.tile_pool(name="pconst", bufs=1))
        pp = es.enter_context(tc.tile_pool(name="pp", bufs=1))

        # ---- stage constants
        fuse_w = pconst.tile([128, 64], BF16)
        nc.sync.dma_start(fuse_w[:], fuse_lhsT.ap())
        conv_w = {}
        for q in ("dy", "dx", "mod"):
            for v in range(6):
                tP = pconst.tile([128, 128], BF16, name=f"cwp_{q}{v}")
                nc.sync.dma_start(tP[:], pw[(q, v)].ap())
                tS = pconst.tile([64, 128], BF16, name=f"cws_{q}{v}")
                nc.sync.dma_start(tS[:], sw[(q, v)].ap())
                conv_w[(q, v)] = (tP, tS)
        col = {}
        for nm in ("bias_dy", "bias_dx", "bias_mod", "xw_s1", "xw_s2"):
            t = pconst.tile([128, 1], F32, name=f"c_{nm}")
            nc.sync.dma_start(t[:], dcols[nm].ap())
            col[nm] = t
        sel_sb = pconst.tile([128, K2 * 128], BF16)
        nc.sync.dma_start(sel_sb[:], sel.ap())
        regsb = {}
        for y, t in ((0, reg0), (1, reg1)):
            r = pconst.tile([128, K2 * 64], BF16, name=f"regsb{y}")
            nc.sync.dma_start(r[:], t.ap())
            regsb[y] = r

        CY, IDXT = {}, {}
        WIDX = pp.tile([128, K2 * 8 * S], I16, name="widx")

        with tc.tile_pool(name="pX", bufs=1) as pX:
            X2 = pp.tile([128, PADH, PADW], BF16, name="X2")

            # =============== phase 0 ===============
            with tc.tile_pool(name="pin", bufs=1) as pin, \
                 tc.tile_pool(name="p0ps", bufs=2, space="PSUM") as p0ps:
                instk = pin.tile([128, NPX], BF16)
                nc.gpsimd.dma_start(instk[0:64, :], x_img.ap())
                nc.gpsimd.dma_start(instk[64:128, :], x_cont.ap())

                # zero only the X2 pad ring: interior rows 2..129 x cols are
                # fully written by the fuse copies below (cols 2:130 on rows
                # 0-63, cols 0:128 on dup rows 64-127)
                nc.vector.memset(X2[:, 0:2, :], 0.0)
                nc.vector.memset(X2[:, PADH - 2:PADH, :], 0.0)
                nc.vector.memset(X2[0:64, 2:PADH - 2, 0:2], 0.0)
                nc.vector.memset(X2[0:64, 2:PADH - 2, 130:PADW], 0.0)
                nc.vector.memset(X2[64:128, 2:PADH - 2, 128:PADW], 0.0)
                zt = pin.tile([128, 2048], BF16)
                nc.vector.memset(zt[:, :], 0.0)
                # zero the pad ring of the column-major token image:
                # y in {0,1,130,131} for all x; x in {0,1} and {129..131}
                for y0 in (0, PADH - 2):
                    for x0 in range(0, PADW, 64):
                        nx = min(64, PADW - x0)
                        nc.sync.dma_start(xtokC3[x0:x0 + nx, y0:y0 + 2, :],
                                          zt[0:nx, 0:256])
                for x0, wd in ((0, 2), (PADW - 3, 3)):
                    for w_ in range(wd):
                        for y0 in range(2, PADH - 2, 64):
                            ny = min(64, PADH - 2 - y0)
                            nc.sync.dma_start(
                                xtokC3[x0 + w_:x0 + w_ + 1, y0:y0 + ny, :]
                                .rearrange("a b e -> (a b) e"),
                                zt[0:ny, 0:128])

                # fuse conv -> X2 rows 0-63 interior
                for c8 in range(NCH):
                    for j in range(NSPL):
                        ps = p0ps.tile([64, SPL], F32, tag="fuseps", bufs=4)
                        off = c8 * CH + j * SPL
                        MM(ps[:], fuse_w[:, :], instk[:, off:off + SPL],
                           start=True, stop=True)
                        i0 = off // W
                        ps3 = ps[:].rearrange("p (a b) -> p a b", b=W)
                        nc.scalar.copy(X2[0:64, 2 + i0:2 + i0 + RSPL, 2:130], ps3)
                        # dup rows hold x_pad shifted +2 cols: same psum data
                        # lands at col 0 (X2[64+c,i,b] = x_pad[c,i,b+2])
                        nc.scalar.copy(X2[64:128, 2 + i0:2 + i0 + RSPL, 0:128], ps3)

                # transposed fuse -> xtok tokens
                stg = pin.tile([128, RPC * 64], BF16, tag="stg", bufs=2)
                for c8 in range(NCH):
                    for r in range(RPC):
                        i = c8 * RPC + r
                        pst = p0ps.tile([128, 64], F32, tag="fuseT", bufs=4)
                        MM(pst[:], instk[:, i * W:(i + 1) * W], fuse_w[:, :],
                           start=True, stop=True)
                        nc.vector.tensor_copy(stg[:, r * 64:(r + 1) * 64], pst[:])
                    rr = c8 * RPC + 2
                    # first halves: token (y, x=2+j)[0:64] = pixel (y, 2+j)
                    nc.sync.dma_start(
                        xtokC3[2:130, rr:rr + RPC, 0:64],
                        stg[:, :].rearrange("p (r e) -> p r e", e=64))
                    # second halves: token (y, x=1+j)[64:128] = pixel (y, 2+j)
                    nc.sync.dma_start(
                        xtokC3[1:129, rr:rr + RPC, 64:128],
                        stg[:, :].rearrange("p (r e) -> p r e", e=64))

    
            # derive 512B pair-row tokens: P copy (even y0), Q copy (odd y0).
            # Column-major xtok makes a y-pair 512B contiguous: one descriptor
            # per output token row.
            for par, npair, base in ((0, NPAIR_P, 0), (1, NPAIR_Q, QBASE)):
                nc.sync.dma_start(
                    xtok2.ap()[base:base + npair * PADW, :]
                    .rearrange("(a b) e -> a b e", b=PADW),
                    xtokC3[:, par:par + 2 * npair, :]
                    .rearrange("x (a c) e -> a x (c e)", c=2))

            # =============== phase A: convs + maps ===============
            # Row-block conv: 4 consecutive output rows (delta) live in the
            # matmul M dim (M = 128 = 4 delta x 32 lanes).  One P+S matmul
            # pair per input row (6 per block).  Chunking: chunk (g, cb) =
            # image rows r with r%4 == cb, (r//4)%2 == g; in-chunk col
            # n = (r//8)*128 + j.
            with tc.tile_pool(name="paps", bufs=4, space="PSUM") as paps, \
                 tc.tile_pool(name="pam", bufs=1) as pam:

                def run_phase_c(g, pcps, pops, pc, pv):
                    nchunk = [0]             # path-mix counter
                    for hh in range(2):      # half-group: chunks (2hh, 2hh+1)
                        outp = pops.tile([128, CH], F32, tag="outp", bufs=1)
                        pend, CLAG = [], 6
                        for k in range(K2):
                            vb = {}
                            for ci2 in range(2):
                                vv = pv.tile([128, 2, CH], BF16, tag="vt",
                                             bufs=6, name=f"vv{ci2}")
                                islot = ((g * K2 + k) * 4 + 2 * hh + ci2) * S
                                nc.gpsimd.dma_gather(
                                    vv[:, :, :], xtok2.ap(),
                                    WIDX[:, islot:islot + S],
                                    num_idxs=CH, num_idxs_reg=CH,
                                    elem_size=256, transpose=True,
                                    single_packet=False)
                                vb[ci2] = vv
                            for y in (0, 1):
                                GW = 1024            # chunk cols (2 psum banks)
                                NH = CH // GW
                                NSUB = GW // SPL     # matmul splits (N<=512)
                                for ci in range(2):
                                    cb = 2 * hh + ci
                                    cy = CY[(g, y)]
                                    cyv = cy[32 * cb:32 * cb + 18, :].rearrange(
                                        "p (a b) -> p b a", b=S)   # [18, S, 16]
                                    for h in range(NH):
                                        wrepp = pcps.tile([128, GW], F32,
                                                          tag="wrepp", bufs=2)
                                        for u in range(NSUB):
                                            q0 = (h * GW + u * SPL) // 16
                                            MM(wrepp[:, u * SPL:(u + 1) * SPL],
                                               sel_sb[32 * cb:32 * cb + 18,
                                                      k * 128:(k + 1) * 128],
                                               cyv[:, q0:q0 + SPL // 16, :],
                                               start=True, stop=True,
                                               tile_position=(32 * cb, 0),
                                               skip_group_check=True)
                                        prd = pc.tile([128, GW], BF16,
                                                      tag="prd", bufs=6)
                                        # mixed path: most chunks copy weights
                                        # via ACT then 2x-mode DVE mult; the
                                        # rest multiply straight out of PSUM
                                        # on DVE (no copy, slower per col)
                                        if nchunk[0] % 4 != 3:
                                            wreps = pc.tile([128, GW], BF16,
                                                            tag="wreps", bufs=4)
                                            nc.scalar.copy(wreps[:], wrepp[:])
                                            nc.vector.tensor_tensor(
                                                prd[:],
                                                vb[ci][:, y, h * GW:(h + 1) * GW],
                                                wreps[:], Op.mult)
                                        else:
                                            nc.vector.tensor_tensor(
                                                prd[:],
                                                vb[ci][:, y, h * GW:(h + 1) * GW],
                                                wrepp[:], Op.mult)
                                        nchunk[0] += 1

                                        def _emit_contr(prd=prd, y=y, k=k,
                                                        ci=ci, h=h, outp=outp,
                                                        GW=GW, NSUB=NSUB):
                                            for u in range(NSUB):
                                                MM(outp[64 * ci:64 * ci + 64,
                                                        h * GW + u * SPL:
                                                        h * GW + (u + 1) * SPL],
                                                   regsb[y][:, k * 64:(k + 1) * 64],
                                                   prd[:, u * SPL:(u + 1) * SPL],
                                                   start=(k == 0 and y == 0),
                                                   stop=(k == K2 - 1 and y == 1),
                                                   skip_group_check=True)
                                        pend.append(_emit_contr)
                                        if len(pend) > CLAG:
                                            pend.pop(0)()
                        for fe in pend:
                            fe()
                        for ci in range(2):
                            cb = 2 * hh + ci
                            outs = pc.tile([64, CH], F32, tag="outs", bufs=2)
                            # out col m = p*S + q <- outp col n = q*16 + p
                            opv = outp[64 * ci:64 * ci + 64, :].rearrange(
                                "p (q a) -> p a q", a=16)       # [64, 16, S]
                            nc.scalar.copy(
                                outs[:].rearrange("p (a q) -> p a q", a=16),
                                opv)
                            # chunk (g, cb) holds rows r = rb*8 + g*4 + cb
                            out4 = out.ap().rearrange(
                                "c (rb e j) -> c rb e j", e=8, j=W)
                            nc.sync.dma_start(
                                out4[:, :, g * 4 + cb, :],
                                outs[:].rearrange("c (rb j) -> c rb j", j=W))

                for g in range(2):
                    qsb = {}
                    for q in ("dy", "dx", "mod"):
                        qs = pam.tile([128, CH], BF16, tag=f"q_{q}",
                                      name=f"qsb_{q}{g}")
                        bias_col = col["bias_dy" if q == "dy"
                                       else "bias_dx" if q == "dx"
                                       else "bias_mod"]
                        for u in range(4):
                            qps = paps.tile([128, 512], F32, tag="convps")
                            # one matmul per (v, P/S) covers 4 row-blocks tt
                            # via a stride-8 row AP (N = 4x128 = 512)
                            base = 1 + g * 4 + 32 * u
                            for v in range(6):
                                tP, tS = conv_w[(q, v)]
                                MM(qps[:, :], tP[:, :],
                                   X2[0:128, base + v:base + v + 25:8, 1:1 + W],
                                   start=(v == 0), stop=False)
                                MM(qps[:, :], tS[:, :],
                                   X2[0:64, base + v:base + v + 25:8, 2:2 + W],
                                   start=False, stop=(v == 5))
                            nc.scalar.activation(
                                qs[:, u * 512:(u + 1) * 512], qps[:],
                                Act.Identity, bias=bias_col[:], scale=1.0)
                        if q == "mod":
                            for cb in range(4):
                                sl = qs[32 * cb:32 * cb + 18, :]
                                nc.scalar.activation(sl, sl, Act.Sigmoid,
                                                     scale=1.0)
                        qsb[q] = qs

                    FY = pam.tile([128, CH], BF16, tag="m1")
                    nc.vector.tensor_scalar(FY[:], qsb["dy"][:], 0.0, None, Op.is_lt)
                    FX = pam.tile([128, CH], BF16, tag="m2")
                    nc.vector.tensor_scalar(FX[:], qsb["dx"][:], 0.0, None, Op.is_lt)
                    RY = pam.tile([128, CH], BF16, tag="m3")
                    nc.vector.tensor_tensor(RY[:], qsb["dy"][:], FY[:], Op.add)
                    RX = pam.tile([128, CH], BF16, tag="m4")
                    nc.vector.tensor_tensor(RX[:], qsb["dx"][:], FX[:], Op.add)
                    XW = pam.tile([128, CH], BF16, tag="m5")
                    nc.vector.tensor_scalar(XW[:], RX[:], col["xw_s1"][:],
                                            col["xw_s2"][:], Op.mult, Op.add)
                    WY0N = pam.tile([128, CH], BF16, tag="m6")
                    nc.vector.scalar_tensor_tensor(WY0N[:], RY[:], 1.0,
                                                   qsb["mod"][:],
                                                   Op.subtract, Op.mult)
                    RYM = pam.tile([128, CH], BF16, tag="m7")
                    nc.vector.tensor_tensor(RYM[:], RY[:], qsb["mod"][:], Op.mult)
                    cy0 = pp.tile([128, CH], BF16, name=f"cy0_{g}")
                    nc.vector.tensor_tensor(cy0[:], WY0N[:], XW[:], Op.mult)
                    cy1 = pp.tile([128, CH], BF16, name=f"cy1_{g}")
                    nc.vector.tensor_tensor(cy1[:], RYM[:], XW[:], Op.mult)
                    CY[(g, 0)], CY[(g, 1)] = cy0, cy1

                    btok0 = pam.tile([128, CH], F32, tag="bt0", bufs=2,
                                     name=f"bt0s_{g}")
                    nc.sync.dma_start(btok0[:], btg[("bt0", g)].ap())
                    btokd = pam.tile([128, CH], F32, tag="btd", bufs=2,
                                     name=f"btds_{g}")
                    nc.sync.dma_start(btokd[:], btg[("btd", g)].ap())
                    T1 = pam.tile([128, CH], F32, tag="m8")
                    nc.vector.tensor_tensor(T1[:], FY[:], btokd[:], Op.mult)
                    TOK0 = pam.tile([128, CH], F32, tag="m9")
                    nc.vector.tensor_tensor(TOK0[:], btok0[:], T1[:],
                                            Op.subtract)
                    T2 = pam.tile([128, CH], F32, tag="m8", name="T2")
                    nc.vector.tensor_tensor(T2[:], TOK0[:], FX[:], Op.subtract)
                    idx0 = pp.tile([128, CH], I16, name=f"idx0_{g}")
                    nc.vector.tensor_copy(idx0[:], T2[:])
                    IDXT[g] = idx0

                    # wrapped indices for this g:
                    # WIDX[y][p, slot*S + s] = IDX[row, p*S + s]
                    for k in range(K2):
                        for cb in range(4):
                            slot = ((g * K2 + k) * 4 + cb) * S
                            sap = IDXT[g][32 * cb + k:32 * cb + k + 1, :]
                            eng = nc.sync if (k + cb) % 2 == 0 else nc.scalar
                            eng.dma_start(
                                WIDX[0:16, slot:slot + S],
                                sap.rearrange("p (a b) -> p a b", b=S))
                    HW_ = K2 * 4 * S
                    for r8 in range(1, 8):
                        nc.sync.dma_start(
                            WIDX[16 * r8:16 * r8 + 16, g * HW_:(g + 1) * HW_],
                            WIDX[0:16, g * HW_:(g + 1) * HW_])

        # =============== phase C: gather / weight / contract ===============
        with tc.tile_pool(name="pcps", bufs=2, space="PSUM") as pcps, \
             tc.tile_pool(name="pops", bufs=1, space="PSUM") as pops, \
             tc.tile_pool(name="pc", bufs=3) as pc, \
             tc.tile_pool(name="pv", bufs=1) as pv:
            for g in range(2):
                run_phase_c(g, pcps, pops, pc, pv)
        es.close()

    nc.compile()
    return nc


# ======================= host-side preparation =======================

def _bf16(x):
    x = np.asarray(x, np.float32)
    u = x.view(np.uint32)
    r = ((u >> 16) + ((u >> 15) & 1)).astype(np.uint16)  # rne-ish
    return r


def _host_consts(w_fuse, w_off, b_off, w_mod, b_mod, w_reg, H=128):
    NPX, NCH, CH, RPC, S, PADH, NTOK, NSPL, SPL = _shape_consts(H)
    import ml_dtypes
    bf = lambda x: np.asarray(x, np.float32).astype(ml_dtypes.bfloat16)

    consts = {}
    wf = np.asarray(w_fuse, np.float32).reshape(64, 128)
    consts["fuse_lhsT"] = bf(np.ascontiguousarray(wf.T))

    w_off = np.asarray(w_off, np.float32).reshape(18, 64, 3, 3)
    w_mod = np.asarray(w_mod, np.float32).reshape(9, 64, 3, 3)

    def qw(q, k):
        return (w_off[2 * k] if q == "dy"
                else w_off[2 * k + 1] if q == "dx" else w_mod[k])

    for q in ("dy", "dx", "mod"):
        for v in range(6):
            P = np.zeros((128, 128), np.float32)
            Sg = np.zeros((64, 128), np.float32)
            for d in range(4):
                ty = v - d
                if ty < 0 or ty > 2:
                    continue
                for m in range(18):
                    k = m % 9
                    P[0:64, d * 32 + m] = qw(q, k)[:, ty, 0]
                    P[64:128, d * 32 + m] = qw(q, k)[:, ty, 2]
                    Sg[0:64, d * 32 + m] = qw(q, k)[:, ty, 1]
            consts[f"pw_{q}_{v}"] = bf(P)
            consts[f"sw_{q}_{v}"] = bf(Sg)

    b_off = np.asarray(b_off, np.float32)
    b_mod = np.asarray(b_mod, np.float32)
    bdy = np.zeros((128, 1), np.float32)
    bdx = np.zeros((128, 1), np.float32)
    bmd = np.zeros((128, 1), np.float32)
    s1 = np.zeros((128, 1), np.float32)
    s2 = np.zeros((128, 1), np.float32)
    for r in range(128):
        rr = r % 32
        if rr < 18:
            k = rr % 9
            bdy[r] = b_off[2 * k]
            bdx[r] = b_off[2 * k + 1]
            bmd[r] = b_mod[k]
        if rr < 9:
            s1[r], s2[r] = -1.0, 1.0
        elif rr < 18:
            s1[r], s2[r] = 1.0, 0.0
    consts["bias_dy"], consts["bias_dx"], consts["bias_mod"] = bdy, bdx, bmd
    consts["xw_s1"], consts["xw_s2"] = s1, s2

    PADH_ = H + 4
    QBASE = (PADH_ // 2) * PADW

    def _pairtok(y0):
        return np.where(y0 % 2 == 0, (y0 // 2) * PADW,
                        QBASE + (y0 // 2) * PADW)

    b0 = np.zeros((128, 2 * CH), np.float32)
    bd = np.zeros((128, 2 * CH), np.float32)
    for r in range(128):
        cb = r // 32
        rr = r % 32
        k = rr % 9 if rr < 18 else 0
        ky, kx = k // 3, k % 3
        for g in range(2):
            cols = np.arange(CH)
            i = (cols // 128) * 8 + g * 4 + cb
            j = cols % 128
            yb = i + 1 + ky
            pt0 = _pairtok(yb) + (j + 1 + kx)
            ptm = _pairtok(yb - 1) + (j + 1 + kx)
            b0[r, g * CH:(g + 1) * CH] = pt0
            bd[r, g * CH:(g + 1) * CH] = pt0 - ptm
    consts["bt0_0"] = np.ascontiguousarray(b0[:, 0:CH])
    consts["bt0_1"] = np.ascontiguousarray(b0[:, CH:])
    consts["btd_0"] = np.ascontiguousarray(bd[:, 0:CH])
    consts["btd_1"] = np.ascontiguousarray(bd[:, CH:])

    selm = np.zeros((128, K2 * 128), np.float32)
    for cb in range(4):
        for k in range(K2):
            selm[32 * cb + k, k * 128:k * 128 + 64] = 1.0
            selm[32 * cb + k + 9, k * 128 + 64:k * 128 + 128] = 1.0
    consts["sel"] = bf(selm)

    w_reg = np.asarray(w_reg, np.float32).reshape(64, 64, 3, 3)
    r0 = np.zeros((128, K2 * 64), np.float32)
    r1 = np.zeros((128, K2 * 64), np.float32)
    for k in range(K2):
        ky, kx = k // 3, k % 3
        blkT = w_reg[:, :, ky, kx].T       # [c, o]
        r0[0:64, k * 64:(k + 1) * 64] = -2.0 * blkT
        r0[64:128, k * 64:(k + 1) * 64] = -2.0 * blkT
        r1[0:64, k * 64:(k + 1) * 64] = 2.0 * blkT
        r1[64:128, k * 64:(k + 1) * 64] = 2.0 * blkT
    consts["reg0"] = bf(r0)
    consts["reg1"] = bf(r1)
    return consts


_NC_CACHE = {}


def kernel(x_img, x_cont, w_fuse, w_off, b_off, w_mod, b_mod, w_reg):
    from concourse.bass_utils import run_bass_kernel_spmd

    H = 128
    B = int(x_img.shape[0])
    NPX = H * W
    if "nc" not in _NC_CACHE:
        _NC_CACHE["nc"] = build_nc(H=H, num_devices=8)
    nc = _NC_CACHE["nc"]

    consts = _host_consts(w_fuse, w_off, b_off, w_mod, b_mod, w_reg, H=H)
    x_img = np.asarray(x_img, np.float32)
    x_cont = np.asarray(x_cont, np.float32)
    in_maps = []
    for b in range(B):
        m = dict(consts)
        m["x_img"] = np.ascontiguousarray(x_img[b].reshape(C, NPX))
        m["x_cont"] = np.ascontiguousarray(x_cont[b].reshape(C, NPX))
        in_maps.append(m)

    res = run_bass_kernel_spmd(nc, in_maps, core_ids=list(range(B)))
    outs = [np.asarray(res.results[b]["out"], np.float32).reshape(C, H, W)
            for b in range(B)]
    return np.stack(outs)

